# revision 9
# baseline (speedup 1.0000x reference)
"""Decoder layer (ExpansionNet_v2) kernel.

Contract: kernel(**inputs) takes FULL unsharded inputs (as produced by
setup_inputs()) and returns the FULL output [512, 20, 512] fp32.

Strategy: pure data parallel over the batch (beam) dim across 8 NeuronCores
(64 batch elements per core), weights replicated. Two Bass/Tile NEFFs run on
the 8 trn2 cores:
  launch A: the five DynamicExpansionBlock linears (cond/key/a/b/sel) on the
            1280 decoder tokens per core plus the cross-attention K/V
            projections on the 9216 encoder tokens per core — all fp32r
            matmuls, token-major outputs.
  launch C: the FeedForward block: h^T = relu(ff1 @ x2^T + b1) kept
            feature-major in SBUF (2048 rows), then ff2 contraction back to
            token-major [1280, 512] — never spilling h to HBM.
The remaining glue (LayerNorms, expansion-attention normalization, cross-MHA
softmax, q/o projections, residuals) runs in fp32 on host. If the device
path is unavailable at call time the kernel falls back to a full-host fp32
implementation.
"""

import os
import numpy as np

D = 512
H = 8
DK = 64
DFF = 2048
NE = 16
BS = 512
L = 20
ENC = 144
EPS = 1e-4
NCORES = 8
BPC = BS // NCORES       # 64 batch elements per core
T1 = BPC * L             # 1280 decoder tokens per core
T2 = BPC * ENC           # 9216 encoder tokens per core

# Per-launch HW exec time (ns) from the last traced run; test.py reads this.
EXEC_NS = {}


def _ln(x, g, b):
    m = x.mean(-1, keepdims=True)
    v = ((x - m) ** 2).mean(-1, keepdims=True)
    return (x - m) / np.sqrt(v + EPS) * g + b


def _lin(x, w, b):
    return x @ w.T + b


# ---------------------------------------------------------------------------
# Launch A: token-major y = x @ W.T for 5 DE weights (decoder tokens) and
# K/V weights (encoder tokens). lhsT = x^T chunks, rhs = W.T chunks.
# ---------------------------------------------------------------------------

def _build_main_kernel():
    import sys
    if "/opt/trn_rl_repo" not in sys.path:
        sys.path.insert(0, "/opt/trn_rl_repo")
    import concourse.bass as bass
    import concourse.tile as tile
    import concourse.mybir as mybir
    from concourse import bacc

    nc = bacc.Bacc("TRN2", target_bir_lowering=False, debug=False)
    x2t_d = nc.dram_tensor("x2t", [D, T1], mybir.dt.float32, kind="ExternalInput").ap()
    cxt_d = nc.dram_tensor("cxt", [D, T2], mybir.dt.bfloat16, kind="ExternalInput").ap()
    wde_d = nc.dram_tensor("wde", [D, 5 * D], mybir.dt.float32, kind="ExternalInput").ap()
    wkv_d = nc.dram_tensor("wkv", [D, 2 * D], mybir.dt.float32, kind="ExternalInput").ap()
    yde_d = nc.dram_tensor("yde", [T1, 5 * D], mybir.dt.bfloat16, kind="ExternalOutput").ap()
    ykv_d = nc.dram_tensor("ykv", [T2, 2 * D], mybir.dt.bfloat16, kind="ExternalOutput").ap()

    with tile.TileContext(nc) as tc:
        with tc.tile_pool(name="wpool", bufs=1) as wpool, \
             tc.tile_pool(name="xin", bufs=3) as xin, \
             tc.tile_pool(name="xr", bufs=3) as xr, \
             tc.tile_pool(name="outp", bufs=6) as outp, \
             tc.tile_pool(name="ps", bufs=4, space="PSUM") as ps:
            wde_f = wpool.tile([128, 4, 5 * D], mybir.dt.float32)
            wkv_f = wpool.tile([128, 4, 2 * D], mybir.dt.float32)
            nc.gpsimd.dma_start(wde_f[:], wde_d.rearrange("(c p) n -> p c n", p=128))
            nc.gpsimd.dma_start(wkv_f[:], wkv_d.rearrange("(c p) n -> p c n", p=128))
            wde_r = wpool.tile([128, 4, 5 * D], mybir.dt.float32r)
            wkv_r = wpool.tile([128, 4, 2 * D], mybir.dt.float32r)
            nc.vector.tensor_copy(wde_r[:], wde_f[:])
            nc.vector.tensor_copy(wkv_r[:], wkv_f[:])

            for src_d, in_dt, nmt, wr, nj, out_d in (
                (x2t_d, mybir.dt.float32, T1 // 128, wde_r, 5, yde_d),
                (cxt_d, mybir.dt.bfloat16, T2 // 128, wkv_r, 2, ykv_d),
            ):
                for m in range(nmt):
                    x_f = xin.tile([128, 4, 128], in_dt, tag="xf")
                    nc.sync.dma_start(
                        x_f[:],
                        src_d[:, m * 128:(m + 1) * 128].rearrange(
                            "(c p) t -> p c t", p=128),
                    )
                    x_r = xr.tile([128, 4, 128], mybir.dt.float32r, tag="xr")
                    nc.vector.tensor_copy(x_r[:], x_f[:])
                    for j in range(nj):
                        acc = ps.tile([128, D], mybir.dt.float32, tag="acc")
                        for c in range(4):
                            nc.tensor.matmul(
                                acc[:], x_r[:, c, :],
                                wr[:, c, j * D:(j + 1) * D],
                                start=(c == 0), stop=(c == 3))
                        o_t = outp.tile([128, D], mybir.dt.bfloat16, tag="ot")
                        nc.vector.tensor_copy(o_t[:], acc[:])
                        nc.sync.dma_start(
                            out_d[m * 128:(m + 1) * 128, j * D:(j + 1) * D],
                            o_t[:])
    nc.compile()
    return nc


# ---------------------------------------------------------------------------
# Launch C: FF block. h^T = relu(ff1 @ x2^T + b1) feature-major [2048, T],
# then y = h @ ff2.T token-major [T, 512] (no bias, no residual — host adds).
# ---------------------------------------------------------------------------

def _build_ff_kernel():
    import sys
    if "/opt/trn_rl_repo" not in sys.path:
        sys.path.insert(0, "/opt/trn_rl_repo")
    import concourse.bass as bass
    import concourse.tile as tile
    import concourse.mybir as mybir
    from concourse import bacc

    nc = bacc.Bacc("TRN2", target_bir_lowering=False, debug=False)
    x2t_d = nc.dram_tensor("x2t", [D, T1], mybir.dt.float32, kind="ExternalInput").ap()
    f1t_d = nc.dram_tensor("f1t", [D, DFF], mybir.dt.float32, kind="ExternalInput").ap()
    f2t_d = nc.dram_tensor("f2t", [DFF, D], mybir.dt.float32, kind="ExternalInput").ap()
    b1_d = nc.dram_tensor("b1", [DFF], mybir.dt.float32, kind="ExternalInput").ap()
    yff_d = nc.dram_tensor("yff", [T1, D], mybir.dt.float32, kind="ExternalOutput").ap()

    NH = DFF // 128  # 16 feature-chunks of the hidden dim

    with tile.TileContext(nc) as tc:
        with tc.tile_pool(name="wpool", bufs=1) as wpool, \
             tc.tile_pool(name="stage", bufs=3) as stage, \
             tc.tile_pool(name="hpool", bufs=1) as hpool, \
             tc.tile_pool(name="hstage", bufs=4) as hstage, \
             tc.tile_pool(name="outp", bufs=4) as outp, \
             tc.tile_pool(name="ps", bufs=4, space="PSUM") as ps:
            b1_t = wpool.tile([128, NH], mybir.dt.float32)
            nc.gpsimd.dma_start(b1_t[:], b1_d.rearrange("(c p) -> p c", p=128))
            f1_r = wpool.tile([128, 4, DFF], mybir.dt.float32r)
            f2_r = wpool.tile([128, NH, D], mybir.dt.float32r)
            x_r = wpool.tile([128, 4, T1], mybir.dt.float32r)
            f1t_c = f1t_d.rearrange("(c p) n -> p c n", p=128)
            f2t_c = f2t_d.rearrange("(c p) n -> p c n", p=128)
            x2t_c = x2t_d.rearrange("(c p) t -> p c t", p=128)
            for c in range(4):
                s_t = stage.tile([128, DFF], mybir.dt.float32, tag="st")
                nc.sync.dma_start(s_t[:], f1t_c[:, c, :])
                nc.vector.tensor_copy(f1_r[:, c, :], s_t[:])
            for hm in range(NH):
                s_t = stage.tile([128, DFF], mybir.dt.float32, tag="st")
                nc.sync.dma_start(s_t[:, :D], f2t_c[:, hm, :])
                nc.vector.tensor_copy(f2_r[:, hm, :], s_t[:, :D])
            for c in range(4):
                s_t = stage.tile([128, DFF], mybir.dt.float32, tag="st")
                nc.sync.dma_start(s_t[:, :T1], x2t_c[:, c, :])
                nc.vector.tensor_copy(x_r[:, c, :], s_t[:, :T1])

            slices = [(0, 512), (512, 512), (1024, 256)]
            for t0, ns in slices:
                h_r = hpool.tile([128, NH, 512], mybir.dt.float32r, tag="hr")
                for hm in range(NH):
                    acc = ps.tile([128, 512], mybir.dt.float32, tag="acc1")
                    for c in range(4):
                        nc.tensor.matmul(
                            acc[:, :ns], f1_r[:, c, hm * 128:(hm + 1) * 128],
                            x_r[:, c, t0:t0 + ns],
                            start=(c == 0), stop=(c == 3))
                    h_f = hstage.tile([128, 512], mybir.dt.float32, tag="hf")
                    nc.scalar.activation(
                        out=h_f[:, :ns], in_=acc[:, :ns],
                        func=mybir.ActivationFunctionType.Relu,
                        bias=b1_t[:, hm:hm + 1], scale=1.0)
                    nc.vector.tensor_copy(h_r[:, hm, :ns], h_f[:, :ns])
                for mt in range(ns // 128):
                    acc2 = ps.tile([128, D], mybir.dt.float32, tag="acc2")
                    for hm in range(NH):
                        nc.tensor.matmul(
                            acc2[:], h_r[:, hm, mt * 128:(mt + 1) * 128],
                            f2_r[:, hm, :],
                            start=(hm == 0), stop=(hm == NH - 1))
                    o_t = outp.tile([128, D], mybir.dt.float32, tag="ot")
                    nc.vector.tensor_copy(o_t[:], acc2[:])
                    nc.sync.dma_start(
                        yff_d[t0 + mt * 128:t0 + (mt + 1) * 128, :], o_t[:])
    nc.compile()
    return nc


_CACHE = {"main": None, "ff": None, "dead": False}


def _run_spmd(nc, in_maps, tag):
    import sys, time
    if "/opt/trn_rl_repo" not in sys.path:
        sys.path.insert(0, "/opt/trn_rl_repo")
    from concourse import bass_utils
    trace = os.environ.get("KERNEL_TRACE", "0") == "1"
    if trace:
        try:
            from antenv.axon_hooks import get_axon_ntff_profile_hook
            trace = get_axon_ntff_profile_hook() is not None
        except Exception:
            trace = False
    t0 = time.time()
    res = bass_utils.run_bass_kernel_spmd(
        nc, in_maps, core_ids=list(range(NCORES)), trace=trace)
    wall_ns = int((time.time() - t0) * 1e9)
    if res.exec_time_ns:
        EXEC_NS[tag] = res.exec_time_ns
    elif os.environ.get("KERNEL_TRACE", "0") == "1":
        # NTFF profiling unavailable under this axon setup: record the
        # launch wall time (includes transfers — an upper bound, not HW ns).
        EXEC_NS[tag] = wall_ns
    return res


def _device_main(x2a, cross_x, g):
    """Launch A for all cores. Returns (cond,key,a,b,sel) each [BS,L,D]
    (bias added) and k,v each [BS,ENC,D] (bias added), or None."""
    try:
        if _CACHE["dead"]:
            return None
        if _CACHE["main"] is None:
            _CACHE["main"] = _build_main_kernel()
        nc = _CACHE["main"]
        wde = np.concatenate(
            [np.ascontiguousarray(g[n].T) for n in
             ("de_cond_w", "de_key_w", "de_a_w", "de_b_w", "de_sel_w")], axis=1)
        wkv = np.concatenate(
            [np.ascontiguousarray(g["wk"].T),
             np.ascontiguousarray(g["wv"].T)], axis=1)
        import ml_dtypes
        bf16 = ml_dtypes.bfloat16
        in_maps = []
        for c in range(NCORES):
            x2c = x2a[c * BPC:(c + 1) * BPC].reshape(T1, D)
            cxc = cross_x[c * BPC:(c + 1) * BPC].reshape(T2, D)
            in_maps.append({
                "x2t": np.ascontiguousarray(x2c.T),
                "cxt": np.ascontiguousarray(cxc.T).astype(bf16),
                "wde": wde, "wkv": wkv,
            })
        res = _run_spmd(nc, in_maps, "main")
        yde = np.concatenate(
            [r["yde"].astype(np.float32).reshape(BPC, L, 5 * D)
             for r in res.results])
        ykv = np.concatenate(
            [r["ykv"].astype(np.float32).reshape(BPC, ENC, 2 * D)
             for r in res.results])
        cond = yde[:, :, 0 * D:1 * D] + g["de_cond_b"]
        key = yde[:, :, 1 * D:2 * D] + g["de_key_b"]
        a_p = yde[:, :, 2 * D:3 * D] + g["de_a_b"]
        b_p = yde[:, :, 3 * D:4 * D] + g["de_b_b"]
        sel = yde[:, :, 4 * D:5 * D] + g["de_sel_b"]
        k = ykv[:, :, :D] + g["wk_b"]
        v = ykv[:, :, D:] + g["wv_b"]
        return cond, key, a_p, b_p, sel, k, v
    except Exception:
        _CACHE["dead"] = True
        return None


def _device_ff(x2c_full, g):
    """Launch C for all cores. Returns y = relu(x2 @ ff1.T + b1) @ ff2.T
    [BS, L, D] (no ff2 bias), or None."""
    try:
        if _CACHE["dead"]:
            return None
        if _CACHE["ff"] is None:
            _CACHE["ff"] = _build_ff_kernel()
        nc = _CACHE["ff"]
        f1t = np.ascontiguousarray(g["ff1_w"].T)
        f2t = np.ascontiguousarray(g["ff2_w"].T)
        b1 = np.ascontiguousarray(g["ff1_b"])
        in_maps = []
        for c in range(NCORES):
            xc = x2c_full[c * BPC:(c + 1) * BPC].reshape(T1, D)
            in_maps.append({
                "x2t": np.ascontiguousarray(xc.T),
                "f1t": f1t, "f2t": f2t, "b1": b1,
            })
        res = _run_spmd(nc, in_maps, "ff")
        y = np.concatenate([r["yff"].reshape(BPC, L, D) for r in res.results])
        return y
    except Exception:
        _CACHE["dead"] = True
        return None


# ---------------------------------------------------------------------------


def kernel(x, cross_x, n_indexes, fw_mask, bw_mask, cross_mask,
           ln1_g, ln1_b, ln2_g, ln2_b, ln3_g, ln3_b,
           de_cond_w, de_cond_b, de_qexp, de_bexp, de_key_w, de_key_b,
           de_a_w, de_a_b, de_b_w, de_b_b, de_sel_w, de_sel_b,
           wq, wq_b, wk, wk_b, wv, wv_b, wo, wo_b,
           ff1_w, ff1_b, ff2_w, ff2_b):
    f32 = np.float32
    x = np.asarray(x, f32)
    cross_x = np.asarray(cross_x, f32)
    n_indexes = np.asarray(n_indexes)
    args = {k2: np.asarray(v2, f32) for k2, v2 in dict(
        ln1_g=ln1_g, ln1_b=ln1_b, ln2_g=ln2_g, ln2_b=ln2_b,
        ln3_g=ln3_g, ln3_b=ln3_b,
        de_cond_w=de_cond_w, de_cond_b=de_cond_b, de_qexp=de_qexp,
        de_bexp=de_bexp, de_key_w=de_key_w, de_key_b=de_key_b,
        de_a_w=de_a_w, de_a_b=de_a_b, de_b_w=de_b_w, de_b_b=de_b_b,
        de_sel_w=de_sel_w, de_sel_b=de_sel_b,
        wq=wq, wq_b=wq_b, wk=wk, wk_b=wk_b, wv=wv, wv_b=wv_b,
        wo=wo, wo_b=wo_b, ff1_w=ff1_w, ff1_b=ff1_b,
        ff2_w=ff2_w, ff2_b=ff2_b).items()}
    g = args

    bs, dec_len, _ = x.shape
    use_device = os.environ.get("KERNEL_NO_DEVICE", "0") != "1"

    # ---- DynamicExpansionBlock ----
    x2 = _ln(x, g["ln1_g"], g["ln1_b"])
    dm = _device_main(x2, cross_x, g) if use_device else None
    if dm is None:
        cond_t = _lin(x2, g["de_cond_w"], g["de_cond_b"])
        xk = _lin(x2, g["de_key_w"], g["de_key_b"])
        a_proj = _lin(x2, g["de_a_w"], g["de_a_b"])
        b_proj = _lin(x2, g["de_b_w"], g["de_b_b"])
        sel_lin = _lin(x2, g["de_sel_w"], g["de_sel_b"])
        k_full = _lin(cross_x, g["wk"], g["wk_b"])
        v_full = _lin(cross_x, g["wv"], g["wv_b"])
    else:
        cond_t, xk, a_proj, b_proj, sel_lin, k_full, v_full = dm

    # Low-rank structure: query[l*NE+e] = qe[e] + cond[l] and
    # bias[l*NE+e] = be[e] + cond[l], so z splits into two small gemms and
    # a_bw @ (a_fw @ A + bias) = (a_bw@a_fw) @ A + Al @ cond + Ae @ be
    # with Al/Ae the mask-group sums of a_bw. Exact same math, no
    # [bs, 320, 512] intermediates.
    qe = g["de_qexp"][n_indexes]                       # [bs, NE, D]
    be = g["de_bexp"][n_indexes]                       # [bs, NE, D]
    scale = f32(1.0 / np.sqrt(D))
    xkT = xk.transpose(0, 2, 1)
    cond_z = (cond_t @ xkT) * scale                    # [bs, L, L]
    qe_z = (qe @ xkT) * scale                          # [bs, NE, L]
    z = (cond_z[:, :, None, :] + qe_z[:, None, :, :]).reshape(
        bs, dec_len * NE, dec_len)
    fwm = fw_mask != 0
    a_fw = np.where(fwm, np.maximum(z, 0.0), 0.0)
    b_fw = np.where(fwm, np.maximum(-z, 0.0), 0.0)
    a_fw /= (a_fw.sum(-1, keepdims=True) + EPS)
    b_fw /= (b_fw.sum(-1, keepdims=True) + EPS)
    zt = z.transpose(0, 2, 1)
    bwm = bw_mask != 0
    a_bw = np.where(bwm, np.maximum(zt, 0.0), 0.0)
    b_bw = np.where(bwm, np.maximum(-zt, 0.0), 0.0)
    a_bw /= (a_bw.sum(-1, keepdims=True) + EPS)
    b_bw /= (b_bw.sum(-1, keepdims=True) + EPS)
    ra = a_bw.reshape(bs, dec_len, dec_len, NE)
    rb = b_bw.reshape(bs, dec_len, dec_len, NE)
    ca = (a_bw @ a_fw) @ a_proj + ra.sum(3) @ cond_t + ra.sum(2) @ be
    cb = (b_bw @ b_fw) @ b_proj + rb.sum(3) @ cond_t + rb.sum(2) @ be
    sel = 1.0 / (1.0 + np.exp(-sel_lin))
    x = x + sel * ca + (1.0 - sel) * cb

    # ---- cross MHA ----
    x2 = _ln(x, g["ln2_g"], g["ln2_b"])
    q = _lin(x2, g["wq"], g["wq_b"]).reshape(bs, dec_len, H, DK).transpose(0, 2, 1, 3)
    enc_len = cross_x.shape[1]
    k = k_full.reshape(bs, enc_len, H, DK).transpose(0, 2, 1, 3)
    v = v_full.reshape(bs, enc_len, H, DK).transpose(0, 2, 1, 3)
    s = np.einsum("bhqd,bhkd->bhqk", q, k, optimize=True) / f32(np.sqrt(DK))
    s = np.where(cross_mask[:, :, :, :] == 1, f32(-1000.0), s)
    s = s - s.max(-1, keepdims=True)
    e = np.exp(s)
    att = e / e.sum(-1, keepdims=True)
    o = np.einsum("bhqk,bhkd->bhqd", att, v,
                  optimize=True).transpose(0, 2, 1, 3).reshape(bs, dec_len, D)
    x = x + _lin(o, g["wo"], g["wo_b"])

    # ---- FeedForward ----
    x2 = _ln(x, g["ln3_g"], g["ln3_b"])
    yff = _device_ff(x2, g) if use_device else None
    if yff is None:
        h = np.maximum(_lin(x2, g["ff1_w"], g["ff1_b"]), 0.0)
        yff = h @ g["ff2_w"].T
    x = x + yff + g["ff2_b"]
    return x.astype(np.float32)


# revision 10
# speedup vs baseline: 1.1166x; 1.1166x over previous
"""Decoder layer (ExpansionNet_v2) kernel.

Contract: kernel(**inputs) takes FULL unsharded inputs (as produced by
setup_inputs()) and returns the FULL output [512, 20, 512] fp32.

Strategy: pure data parallel over the batch (beam) dim across 8 NeuronCores
(64 batch elements per core), weights replicated. Two Bass/Tile NEFFs run on
the 8 trn2 cores:
  launch A: the five DynamicExpansionBlock linears (cond/key/a/b/sel) on the
            1280 decoder tokens per core plus the cross-attention K/V
            projections on the 9216 encoder tokens per core — all fp32r
            matmuls, token-major outputs.
  launch C: the FeedForward block: h^T = relu(ff1 @ x2^T + b1) kept
            feature-major in SBUF (2048 rows), then ff2 contraction back to
            token-major [1280, 512] — never spilling h to HBM.
The remaining glue (LayerNorms, expansion-attention normalization, cross-MHA
softmax, q/o projections, residuals) runs in fp32 on host. If the device
path is unavailable at call time the kernel falls back to a full-host fp32
implementation.
"""

import os
import numpy as np

D = 512
H = 8
DK = 64
DFF = 2048
NE = 16
BS = 512
L = 20
ENC = 144
EPS = 1e-4
NCORES = 8
BPC = BS // NCORES       # 64 batch elements per core
T1 = BPC * L             # 1280 decoder tokens per core
T2 = BPC * ENC           # 9216 encoder tokens per core

# Per-launch HW exec time (ns) from the last traced run; test.py reads this.
EXEC_NS = {}


def _ln(x, g, b):
    m = x.mean(-1, keepdims=True)
    v = ((x - m) ** 2).mean(-1, keepdims=True)
    return (x - m) / np.sqrt(v + EPS) * g + b


def _lin(x, w, b):
    return x @ w.T + b


# ---------------------------------------------------------------------------
# Launch A: token-major y = x @ W.T for 5 DE weights (decoder tokens) and
# K/V weights (encoder tokens). lhsT = x^T chunks, rhs = W.T chunks.
# ---------------------------------------------------------------------------

def _build_main_kernel():
    import sys
    if "/opt/trn_rl_repo" not in sys.path:
        sys.path.insert(0, "/opt/trn_rl_repo")
    import concourse.bass as bass
    import concourse.tile as tile
    import concourse.mybir as mybir
    from concourse import bacc

    nc = bacc.Bacc("TRN2", target_bir_lowering=False, debug=False)
    x2t_d = nc.dram_tensor("x2t", [D, T1], mybir.dt.bfloat16, kind="ExternalInput").ap()
    cxt_d = nc.dram_tensor("cxt", [D, T2], mybir.dt.bfloat16, kind="ExternalInput").ap()
    wde_d = nc.dram_tensor("wde", [D, 5 * D], mybir.dt.bfloat16, kind="ExternalInput").ap()
    wkv_d = nc.dram_tensor("wkv", [D, 2 * D], mybir.dt.bfloat16, kind="ExternalInput").ap()
    yde_d = nc.dram_tensor("yde", [T1, 5 * D], mybir.dt.bfloat16, kind="ExternalOutput").ap()
    ykv_d = nc.dram_tensor("ykv", [T2, 2 * D], mybir.dt.bfloat16, kind="ExternalOutput").ap()

    with tile.TileContext(nc) as tc:
        with tc.tile_pool(name="wpool", bufs=1) as wpool, \
             tc.tile_pool(name="xin", bufs=3) as xin, \
             tc.tile_pool(name="xr", bufs=3) as xr, \
             tc.tile_pool(name="outp", bufs=6) as outp, \
             tc.tile_pool(name="ps", bufs=4, space="PSUM") as ps:
            wde_f = wpool.tile([128, 4, 5 * D], mybir.dt.bfloat16)
            wkv_f = wpool.tile([128, 4, 2 * D], mybir.dt.bfloat16)
            nc.gpsimd.dma_start(wde_f[:], wde_d.rearrange("(c p) n -> p c n", p=128))
            nc.gpsimd.dma_start(wkv_f[:], wkv_d.rearrange("(c p) n -> p c n", p=128))
            wde_r = wpool.tile([128, 4, 5 * D], mybir.dt.float32r)
            wkv_r = wpool.tile([128, 4, 2 * D], mybir.dt.float32r)
            nc.vector.tensor_copy(wde_r[:], wde_f[:])
            nc.vector.tensor_copy(wkv_r[:], wkv_f[:])

            for src_d, in_dt, nmt, wr, nj, out_d in (
                (x2t_d, mybir.dt.bfloat16, T1 // 128, wde_r, 5, yde_d),
                (cxt_d, mybir.dt.bfloat16, T2 // 128, wkv_r, 2, ykv_d),
            ):
                for m in range(nmt):
                    x_f = xin.tile([128, 4, 128], in_dt, tag="xf")
                    nc.sync.dma_start(
                        x_f[:],
                        src_d[:, m * 128:(m + 1) * 128].rearrange(
                            "(c p) t -> p c t", p=128),
                    )
                    x_r = xr.tile([128, 4, 128], mybir.dt.float32r, tag="xr")
                    nc.vector.tensor_copy(x_r[:], x_f[:])
                    for j in range(nj):
                        acc = ps.tile([128, D], mybir.dt.float32, tag="acc")
                        for c in range(4):
                            nc.tensor.matmul(
                                acc[:], x_r[:, c, :],
                                wr[:, c, j * D:(j + 1) * D],
                                start=(c == 0), stop=(c == 3))
                        o_t = outp.tile([128, D], mybir.dt.bfloat16, tag="ot")
                        nc.vector.tensor_copy(o_t[:], acc[:])
                        nc.sync.dma_start(
                            out_d[m * 128:(m + 1) * 128, j * D:(j + 1) * D],
                            o_t[:])
    nc.compile()
    return nc


# ---------------------------------------------------------------------------
# Launch C: FF block. h^T = relu(ff1 @ x2^T + b1) feature-major [2048, T],
# then y = h @ ff2.T token-major [T, 512] (no bias, no residual — host adds).
# ---------------------------------------------------------------------------

def _build_ff_kernel():
    import sys
    if "/opt/trn_rl_repo" not in sys.path:
        sys.path.insert(0, "/opt/trn_rl_repo")
    import concourse.bass as bass
    import concourse.tile as tile
    import concourse.mybir as mybir
    from concourse import bacc

    nc = bacc.Bacc("TRN2", target_bir_lowering=False, debug=False)
    x2t_d = nc.dram_tensor("x2t", [D, T1], mybir.dt.bfloat16, kind="ExternalInput").ap()
    f1t_d = nc.dram_tensor("f1t", [D, DFF], mybir.dt.bfloat16, kind="ExternalInput").ap()
    f2t_d = nc.dram_tensor("f2t", [DFF, D], mybir.dt.bfloat16, kind="ExternalInput").ap()
    b1_d = nc.dram_tensor("b1", [DFF], mybir.dt.float32, kind="ExternalInput").ap()
    yff_d = nc.dram_tensor("yff", [T1, D], mybir.dt.bfloat16, kind="ExternalOutput").ap()

    NH = DFF // 128  # 16 feature-chunks of the hidden dim

    with tile.TileContext(nc) as tc:
        with tc.tile_pool(name="wpool", bufs=1) as wpool, \
             tc.tile_pool(name="stage", bufs=3) as stage, \
             tc.tile_pool(name="hpool", bufs=1) as hpool, \
             tc.tile_pool(name="hstage", bufs=4) as hstage, \
             tc.tile_pool(name="outp", bufs=4) as outp, \
             tc.tile_pool(name="ps", bufs=4, space="PSUM") as ps:
            b1_t = wpool.tile([128, NH], mybir.dt.float32)
            nc.gpsimd.dma_start(b1_t[:], b1_d.rearrange("(c p) -> p c", p=128))
            f1_r = wpool.tile([128, 4, DFF], mybir.dt.float32r)
            f2_r = wpool.tile([128, NH, D], mybir.dt.float32r)
            x_r = wpool.tile([128, 4, T1], mybir.dt.float32r)
            f1t_c = f1t_d.rearrange("(c p) n -> p c n", p=128)
            f2t_c = f2t_d.rearrange("(c p) n -> p c n", p=128)
            x2t_c = x2t_d.rearrange("(c p) t -> p c t", p=128)
            for c in range(4):
                s_t = stage.tile([128, DFF], mybir.dt.bfloat16, tag="st")
                nc.sync.dma_start(s_t[:], f1t_c[:, c, :])
                nc.vector.tensor_copy(f1_r[:, c, :], s_t[:])
            for hm in range(NH):
                s_t = stage.tile([128, DFF], mybir.dt.bfloat16, tag="st")
                nc.sync.dma_start(s_t[:, :D], f2t_c[:, hm, :])
                nc.vector.tensor_copy(f2_r[:, hm, :], s_t[:, :D])
            for c in range(4):
                s_t = stage.tile([128, DFF], mybir.dt.bfloat16, tag="st")
                nc.sync.dma_start(s_t[:, :T1], x2t_c[:, c, :])
                nc.vector.tensor_copy(x_r[:, c, :], s_t[:, :T1])

            slices = [(0, 512), (512, 512), (1024, 256)]
            for t0, ns in slices:
                h_r = hpool.tile([128, NH, 512], mybir.dt.float32r, tag="hr")
                for hm in range(NH):
                    acc = ps.tile([128, 512], mybir.dt.float32, tag="acc1")
                    for c in range(4):
                        nc.tensor.matmul(
                            acc[:, :ns], f1_r[:, c, hm * 128:(hm + 1) * 128],
                            x_r[:, c, t0:t0 + ns],
                            start=(c == 0), stop=(c == 3))
                    h_f = hstage.tile([128, 512], mybir.dt.float32, tag="hf")
                    nc.scalar.activation(
                        out=h_f[:, :ns], in_=acc[:, :ns],
                        func=mybir.ActivationFunctionType.Relu,
                        bias=b1_t[:, hm:hm + 1], scale=1.0)
                    nc.vector.tensor_copy(h_r[:, hm, :ns], h_f[:, :ns])
                for mt in range(ns // 128):
                    acc2 = ps.tile([128, D], mybir.dt.float32, tag="acc2")
                    for hm in range(NH):
                        nc.tensor.matmul(
                            acc2[:], h_r[:, hm, mt * 128:(mt + 1) * 128],
                            f2_r[:, hm, :],
                            start=(hm == 0), stop=(hm == NH - 1))
                    o_t = outp.tile([128, D], mybir.dt.bfloat16, tag="ot")
                    nc.vector.tensor_copy(o_t[:], acc2[:])
                    nc.sync.dma_start(
                        yff_d[t0 + mt * 128:t0 + (mt + 1) * 128, :], o_t[:])
    nc.compile()
    return nc


_CACHE = {"main": None, "ff": None, "dead": False}


def _run_spmd(nc, in_maps, tag):
    import sys, time
    if "/opt/trn_rl_repo" not in sys.path:
        sys.path.insert(0, "/opt/trn_rl_repo")
    from concourse import bass_utils
    trace = os.environ.get("KERNEL_TRACE", "0") == "1"
    if trace:
        try:
            from antenv.axon_hooks import get_axon_ntff_profile_hook
            trace = get_axon_ntff_profile_hook() is not None
        except Exception:
            trace = False
    t0 = time.time()
    res = bass_utils.run_bass_kernel_spmd(
        nc, in_maps, core_ids=list(range(NCORES)), trace=trace)
    wall_ns = int((time.time() - t0) * 1e9)
    if res.exec_time_ns:
        EXEC_NS[tag] = res.exec_time_ns
    elif os.environ.get("KERNEL_TRACE", "0") == "1":
        # NTFF profiling unavailable under this axon setup: record the
        # launch wall time (includes transfers — an upper bound, not HW ns).
        EXEC_NS[tag] = wall_ns
    return res


def _device_main(x2a, cross_x, g):
    """Launch A for all cores. Returns (cond,key,a,b,sel) each [BS,L,D]
    (bias added) and k,v each [BS,ENC,D] (bias added), or None."""
    try:
        if _CACHE["dead"]:
            return None
        if _CACHE["main"] is None:
            _CACHE["main"] = _build_main_kernel()
        nc = _CACHE["main"]
        import ml_dtypes as _mld
        _bf = _mld.bfloat16
        wde = np.concatenate(
            [np.ascontiguousarray(g[n].T) for n in
             ("de_cond_w", "de_key_w", "de_a_w", "de_b_w", "de_sel_w")],
            axis=1).astype(_bf)
        wkv = np.concatenate(
            [np.ascontiguousarray(g["wk"].T),
             np.ascontiguousarray(g["wv"].T)], axis=1).astype(_bf)
        import ml_dtypes
        bf16 = ml_dtypes.bfloat16
        in_maps = []
        for c in range(NCORES):
            x2c = x2a[c * BPC:(c + 1) * BPC].reshape(T1, D)
            cxc = cross_x[c * BPC:(c + 1) * BPC].reshape(T2, D)
            in_maps.append({
                "x2t": np.ascontiguousarray(x2c.T).astype(bf16),
                "cxt": np.ascontiguousarray(cxc.T).astype(bf16),
                "wde": wde, "wkv": wkv,
            })
        res = _run_spmd(nc, in_maps, "main")
        yde = np.concatenate(
            [r["yde"].astype(np.float32).reshape(BPC, L, 5 * D)
             for r in res.results])
        ykv = np.concatenate(
            [r["ykv"].astype(np.float32).reshape(BPC, ENC, 2 * D)
             for r in res.results])
        cond = yde[:, :, 0 * D:1 * D] + g["de_cond_b"]
        key = yde[:, :, 1 * D:2 * D] + g["de_key_b"]
        a_p = yde[:, :, 2 * D:3 * D] + g["de_a_b"]
        b_p = yde[:, :, 3 * D:4 * D] + g["de_b_b"]
        sel = yde[:, :, 4 * D:5 * D] + g["de_sel_b"]
        k = ykv[:, :, :D] + g["wk_b"]
        v = ykv[:, :, D:] + g["wv_b"]
        return cond, key, a_p, b_p, sel, k, v
    except Exception:
        _CACHE["dead"] = True
        return None


def _device_ff(x2c_full, g):
    """Launch C for all cores. Returns y = relu(x2 @ ff1.T + b1) @ ff2.T
    [BS, L, D] (no ff2 bias), or None."""
    try:
        if _CACHE["dead"]:
            return None
        if _CACHE["ff"] is None:
            _CACHE["ff"] = _build_ff_kernel()
        nc = _CACHE["ff"]
        import ml_dtypes
        bf16 = ml_dtypes.bfloat16
        f1t = np.ascontiguousarray(g["ff1_w"].T).astype(bf16)
        f2t = np.ascontiguousarray(g["ff2_w"].T).astype(bf16)
        b1 = np.ascontiguousarray(g["ff1_b"])
        in_maps = []
        for c in range(NCORES):
            xc = x2c_full[c * BPC:(c + 1) * BPC].reshape(T1, D)
            in_maps.append({
                "x2t": np.ascontiguousarray(xc.T).astype(bf16),
                "f1t": f1t, "f2t": f2t, "b1": b1,
            })
        res = _run_spmd(nc, in_maps, "ff")
        y = np.concatenate([r["yff"].astype(np.float32).reshape(BPC, L, D)
                            for r in res.results])
        return y
    except Exception:
        _CACHE["dead"] = True
        return None


# ---------------------------------------------------------------------------


def kernel(x, cross_x, n_indexes, fw_mask, bw_mask, cross_mask,
           ln1_g, ln1_b, ln2_g, ln2_b, ln3_g, ln3_b,
           de_cond_w, de_cond_b, de_qexp, de_bexp, de_key_w, de_key_b,
           de_a_w, de_a_b, de_b_w, de_b_b, de_sel_w, de_sel_b,
           wq, wq_b, wk, wk_b, wv, wv_b, wo, wo_b,
           ff1_w, ff1_b, ff2_w, ff2_b):
    f32 = np.float32
    x = np.asarray(x, f32)
    cross_x = np.asarray(cross_x, f32)
    n_indexes = np.asarray(n_indexes)
    args = {k2: np.asarray(v2, f32) for k2, v2 in dict(
        ln1_g=ln1_g, ln1_b=ln1_b, ln2_g=ln2_g, ln2_b=ln2_b,
        ln3_g=ln3_g, ln3_b=ln3_b,
        de_cond_w=de_cond_w, de_cond_b=de_cond_b, de_qexp=de_qexp,
        de_bexp=de_bexp, de_key_w=de_key_w, de_key_b=de_key_b,
        de_a_w=de_a_w, de_a_b=de_a_b, de_b_w=de_b_w, de_b_b=de_b_b,
        de_sel_w=de_sel_w, de_sel_b=de_sel_b,
        wq=wq, wq_b=wq_b, wk=wk, wk_b=wk_b, wv=wv, wv_b=wv_b,
        wo=wo, wo_b=wo_b, ff1_w=ff1_w, ff1_b=ff1_b,
        ff2_w=ff2_w, ff2_b=ff2_b).items()}
    g = args

    bs, dec_len, _ = x.shape
    use_device = os.environ.get("KERNEL_NO_DEVICE", "0") != "1"

    # ---- DynamicExpansionBlock ----
    x2 = _ln(x, g["ln1_g"], g["ln1_b"])
    dm = _device_main(x2, cross_x, g) if use_device else None
    if dm is None:
        cond_t = _lin(x2, g["de_cond_w"], g["de_cond_b"])
        xk = _lin(x2, g["de_key_w"], g["de_key_b"])
        a_proj = _lin(x2, g["de_a_w"], g["de_a_b"])
        b_proj = _lin(x2, g["de_b_w"], g["de_b_b"])
        sel_lin = _lin(x2, g["de_sel_w"], g["de_sel_b"])
        k_full = _lin(cross_x, g["wk"], g["wk_b"])
        v_full = _lin(cross_x, g["wv"], g["wv_b"])
    else:
        cond_t, xk, a_proj, b_proj, sel_lin, k_full, v_full = dm

    # Low-rank structure: query[l*NE+e] = qe[e] + cond[l] and
    # bias[l*NE+e] = be[e] + cond[l], so z splits into two small gemms and
    # a_bw @ (a_fw @ A + bias) = (a_bw@a_fw) @ A + Al @ cond + Ae @ be
    # with Al/Ae the mask-group sums of a_bw. Exact same math, no
    # [bs, 320, 512] intermediates.
    qe = g["de_qexp"][n_indexes]                       # [bs, NE, D]
    be = g["de_bexp"][n_indexes]                       # [bs, NE, D]
    scale = f32(1.0 / np.sqrt(D))
    xkT = xk.transpose(0, 2, 1)
    cond_z = (cond_t @ xkT) * scale                    # [bs, L, L]
    qe_z = (qe @ xkT) * scale                          # [bs, NE, L]
    z = (cond_z[:, :, None, :] + qe_z[:, None, :, :]).reshape(
        bs, dec_len * NE, dec_len)
    fwm = fw_mask != 0
    a_fw = np.where(fwm, np.maximum(z, 0.0), 0.0)
    b_fw = np.where(fwm, np.maximum(-z, 0.0), 0.0)
    a_fw /= (a_fw.sum(-1, keepdims=True) + EPS)
    b_fw /= (b_fw.sum(-1, keepdims=True) + EPS)
    zt = z.transpose(0, 2, 1)
    bwm = bw_mask != 0
    a_bw = np.where(bwm, np.maximum(zt, 0.0), 0.0)
    b_bw = np.where(bwm, np.maximum(-zt, 0.0), 0.0)
    a_bw /= (a_bw.sum(-1, keepdims=True) + EPS)
    b_bw /= (b_bw.sum(-1, keepdims=True) + EPS)
    ra = a_bw.reshape(bs, dec_len, dec_len, NE)
    rb = b_bw.reshape(bs, dec_len, dec_len, NE)
    ca = (a_bw @ a_fw) @ a_proj + ra.sum(3) @ cond_t + ra.sum(2) @ be
    cb = (b_bw @ b_fw) @ b_proj + rb.sum(3) @ cond_t + rb.sum(2) @ be
    sel = 1.0 / (1.0 + np.exp(-sel_lin))
    x = x + sel * ca + (1.0 - sel) * cb

    # ---- cross MHA ----
    x2 = _ln(x, g["ln2_g"], g["ln2_b"])
    q = _lin(x2, g["wq"], g["wq_b"]).reshape(bs, dec_len, H, DK).transpose(0, 2, 1, 3)
    enc_len = cross_x.shape[1]
    k = k_full.reshape(bs, enc_len, H, DK).transpose(0, 2, 1, 3)
    v = v_full.reshape(bs, enc_len, H, DK).transpose(0, 2, 1, 3)
    s = np.einsum("bhqd,bhkd->bhqk", q, k, optimize=True) / f32(np.sqrt(DK))
    s = np.where(cross_mask[:, :, :, :] == 1, f32(-1000.0), s)
    s = s - s.max(-1, keepdims=True)
    e = np.exp(s)
    att = e / e.sum(-1, keepdims=True)
    o = np.einsum("bhqk,bhkd->bhqd", att, v,
                  optimize=True).transpose(0, 2, 1, 3).reshape(bs, dec_len, D)
    x = x + _lin(o, g["wo"], g["wo_b"])

    # ---- FeedForward ----
    x2 = _ln(x, g["ln3_g"], g["ln3_b"])
    yff = _device_ff(x2, g) if use_device else None
    if yff is None:
        h = np.maximum(_lin(x2, g["ff1_w"], g["ff1_b"]), 0.0)
        yff = h @ g["ff2_w"].T
    x = x + yff + g["ff2_b"]
    return x.astype(np.float32)
